# revision 1
# baseline (speedup 1.0000x reference)
"""LoRA self-attention Trainium2 kernel, 8-way head/tensor parallel.

Sharding: core c owns heads 2c, 2c+1 (= channels 128c..128c+128) for the
QKV projections and attention; the output projection is token-sharded
(core c computes all 1024 output channels for tokens 512c..512c+512)
after an AllToAll exchange of the attention output.

All matmuls run in fp32r (full-rate fp32 on the PE array, ~1.6e-4 rel err).
Layouts are transposed ([channel, token]) end-to-end so every matmul has
its contraction dim on partitions with free dim 512:
  QT/KT = W @ x.T via lhsT=W.T tiles, rhs=xT tiles
  energyT[k,q] = lhsT=KT, rhs=QT  (per head, contraction=64; the two heads
    run concurrently in distinct PE row groups)
  P = exp(energyT/8) on ACT straight out of PSUM (no max subtraction:
    |energy|<~8 so exp is safe in fp32)
  attT[d,q] = lhsT=V_aug, rhs=P with a concurrent M=1 ones-column matmul
    in PE col group 64 producing the softmax denominators for free
  yT = lhsT=Wo.T, rhs=attT(from AllToAll)
LoRA terms accumulate into the same PSUM groups (A-projections packed into
PE col groups 0/32/64 and run concurrently; scale 2.0 folded into B).
"""
import sys

for p in ("/opt/trn_rl_repo",):
    if p not in sys.path:
        sys.path.append(p)

import numpy as np

import concourse.bass as bass  # noqa: F401
import concourse.tile as tile
from concourse import bacc, mybir
from concourse import bass_utils

N_CORES = 8
EMBED = 1024
HEADS = 16
HD = 64            # head dim
RANK = 8
NB = 2             # batch
S = 2048           # seq len
T = NB * S         # 4096 tokens
CH = EMBED // N_CORES  # 128 channels (2 heads) per core
FP = mybir.dt.float32
FPR = mybir.dt.float32r
AF = mybir.ActivationFunctionType

_CACHE: dict = {}


def _build(local_only=False):
    nc = bacc.Bacc("TRN2", target_bir_lowering=False, debug=False,
                   enable_asserts=False, num_devices=N_CORES)
    # ---- DRAM I/O (per-core) ----
    xT = nc.dram_tensor("xT", [EMBED, T], FP, kind="ExternalInput").ap()
    wqT = nc.dram_tensor("wqT", [EMBED, CH], FP, kind="ExternalInput").ap()
    wkT = nc.dram_tensor("wkT", [EMBED, CH], FP, kind="ExternalInput").ap()
    wvT = nc.dram_tensor("wvT", [EMBED, CH], FP, kind="ExternalInput").ap()
    atQKV = nc.dram_tensor("atQKV", [EMBED, 96], FP, kind="ExternalInput").ap()
    Bqkv = nc.dram_tensor("Bqkv", [128, CH], FP, kind="ExternalInput").ap()
    bias3 = nc.dram_tensor("bias3", [CH, 3], FP, kind="ExternalInput").ap()
    ident = nc.dram_tensor("ident", [128, 128], FP, kind="ExternalInput").ap()
    woT = nc.dram_tensor("woT", [EMBED, EMBED], FP, kind="ExternalInput").ap()
    aoT = nc.dram_tensor("aoT", [EMBED, 32], FP, kind="ExternalInput").ap()
    boT2 = nc.dram_tensor("boT2", [RANK, EMBED], FP, kind="ExternalInput").ap()
    bom = nc.dram_tensor("bom", [128, 8], FP, kind="ExternalInput").ap()
    ones64 = nc.dram_tensor("ones64", [128, 64], FP, kind="ExternalInput").ap()
    Y = nc.dram_tensor("Y", [EMBED, 512], FP, kind="ExternalOutput").ap()

    NE = EMBED // 128  # 8 contraction tiles
    NJ = T // 512      # 8 token tiles

    with tile.TileContext(nc) as tc, \
         nc.allow_low_precision(reason="fp32r rounding is intentional"):
        with tc.tile_pool(name="const", bufs=1) as cpool, \
             tc.tile_pool(name="big", bufs=1) as bigpool, \
             tc.tile_pool(name="dram", bufs=1, space="DRAM") as dram:

            # ---- resident weights ----
            wq_sb = [cpool.tile([128, CH], FPR, tag=f"wq{e}", name=f"wq{e}") for e in range(NE)]
            wk_sb = [cpool.tile([128, CH], FPR, tag=f"wk{e}", name=f"wk{e}") for e in range(NE)]
            wv_sb = [cpool.tile([128, CH], FPR, tag=f"wv{e}", name=f"wv{e}") for e in range(NE)]
            at_sb = [cpool.tile([128, 96], FPR, tag=f"at{e}", name=f"at{e}") for e in range(NE)]
            for e in range(NE):
                sl = slice(e * 128, (e + 1) * 128)
                nc.sync.dma_start(wq_sb[e][:], wqT[sl, :].bitcast(FPR))
                nc.sync.dma_start(wk_sb[e][:], wkT[sl, :].bitcast(FPR))
                nc.sync.dma_start(wv_sb[e][:], wvT[sl, :].bitcast(FPR))
                nc.sync.dma_start(at_sb[e][:], atQKV[sl, :].bitcast(FPR))
            B_sb = cpool.tile([128, CH], FPR, tag="Bqkv")
            nc.sync.dma_start(B_sb[:], Bqkv.bitcast(FPR))
            bias_sb = cpool.tile([CH, 3], FP, tag="bias3")
            nc.sync.dma_start(bias_sb[:], bias3)
            id_sb = cpool.tile([128, 128], FPR, tag="ident")
            nc.sync.dma_start(id_sb[:], ident.bitcast(FPR))
            ones_sb = cpool.tile([128, 64], FPR, tag="ones")
            nc.sync.dma_start(ones_sb[:], ones64.bitcast(FPR))

            # ---- resident activations ----
            QT_sb = bigpool.tile([CH, T], FPR, tag="QT")
            KT_sb = bigpool.tile([CH, T], FPR, tag="KT")
            VT_sb = bigpool.tile([CH, T], FPR, tag="VT")
            # V in [token, ch] layout, 32 strips of [128, 130]:
            # cols [s*130+h*65 : +64] = V head h, col [s*130+h*65+64] = ones
            V_sb = bigpool.tile([128, 32 * 130], FPR, tag="Vaug")
            attT0 = bigpool.tile([HD, T], FP, tag="attT0")
            attT1 = bigpool.tile([HD, T], FP, tag="attT1")

            # ================= Phase A: QKV projections =================
            with tc.tile_pool(name="psA", bufs=2, space="PSUM") as psA, \
                 tc.tile_pool(name="xt", bufs=2) as xpool, \
                 tc.tile_pool(name="zsb", bufs=2) as zpool:
                for j in range(NJ):
                    t0 = j * 512
                    xt = []
                    for e in range(NE):
                        xte = xpool.tile([128, 512], FPR, tag=f"xt{e}", name=f"xt{e}")
                        nc.sync.dma_start(
                            xte[:], xT[e * 128:(e + 1) * 128, t0:t0 + 512].bitcast(FPR))
                        xt.append(xte)
                    pq = psA.tile([CH, 512], FP, tag="q")
                    pk = psA.tile([CH, 512], FP, tag="k")
                    pv = psA.tile([CH, 512], FP, tag="v")
                    pz = psA.tile([128, 512], FP, tag="z")
                    for e in range(NE):
                        nc.tensor.matmul(pq[:], wq_sb[e][:], xt[e][:],
                                         start=(e == 0), stop=False)
                        nc.tensor.matmul(pk[:], wk_sb[e][:], xt[e][:],
                                         start=(e == 0), stop=False)
                        nc.tensor.matmul(pv[:], wv_sb[e][:], xt[e][:],
                                         start=(e == 0), stop=False)
                        # LoRA A projections packed at partition bases 0/32/64
                        nc.tensor.matmul(pz[0:96, :], at_sb[e][:], xt[e][:],
                                         start=(e == 0), stop=(e == NE - 1))
                    z_sb = zpool.tile([128, 512], FPR, tag="z_sb")
                    for p0 in (0, 32, 64):
                        nc.vector.tensor_copy(z_sb[p0:p0 + 8, :], pz[p0:p0 + 8, :])
                    nc.tensor.matmul(pq[:], B_sb[0:8, :], z_sb[0:8, :],
                                     start=False, stop=True)
                    nc.tensor.matmul(pk[:], B_sb[32:40, :], z_sb[32:40, :],
                                     start=False, stop=True)
                    nc.tensor.matmul(pv[:], B_sb[64:72, :], z_sb[64:72, :],
                                     start=False, stop=True)
                    nc.scalar.activation(QT_sb[:, t0:t0 + 512], pq[:],
                                         AF.Identity, bias=bias_sb[:, 0:1])
                    nc.scalar.activation(KT_sb[:, t0:t0 + 512], pk[:],
                                         AF.Identity, bias=bias_sb[:, 1:2])
                    nc.scalar.activation(VT_sb[:, t0:t0 + 512], pv[:],
                                         AF.Identity, bias=bias_sb[:, 2:3])

            # ============ Phases B+C: V transpose + attention ============
            with tc.tile_pool(name="psBC", bufs=2, space="PSUM") as psBC, \
                 tc.tile_pool(name="pt", bufs=4) as ptpool, \
                 tc.tile_pool(name="rs", bufs=2) as rpool:
                # V^T -> V via PE transpose; ones columns via memset
                v_ones = V_sb.rearrange("p (s c) -> p s c", c=65)[:, :, 64]
                nc.sync.dma_start(v_ones, ones64.bitcast(FPR))
                for t in range(32):
                    trp = psBC.tile([128, 128], FPR, tag="tr", bufs=1)
                    nc.tensor.transpose(trp[:], VT_sb[:, t * 128:(t + 1) * 128],
                                        id_sb[:])
                    base = t * 130
                    nc.vector.tensor_copy(V_sb[:, base:base + 64], trp[:, 0:64])
                    nc.vector.tensor_copy(V_sb[:, base + 65:base + 129],
                                          trp[:, 64:128])

                for n in range(NB):
                    for jq in range(4):
                        q0 = n * S + jq * 512
                        for h in range(2):
                            hs = slice(h * HD, (h + 1) * HD)
                            po = psBC.tile([64, 512], FP, tag="o", bufs=1)
                            pd = psBC.tile([64, 512], FP, tag="d", bufs=1)
                            for g in range(8):
                                pe = psBC.tile([128, 1024], FP, tag="e")
                                for m in range(2):
                                    ik = 2 * g + m
                                    k0 = n * S + ik * 128
                                    nc.tensor.matmul(
                                        pe[:, m * 512:(m + 1) * 512],
                                        KT_sb[hs, k0:k0 + 128],
                                        QT_sb[hs, q0:q0 + 512],
                                        start=True, stop=True)
                                pt = ptpool.tile([128, 1024], FPR, tag="pt")
                                nc.scalar.activation(pt[:], pe[:], AF.Exp,
                                                     scale=0.125)
                                for m in range(2):
                                    ik = 2 * g + m
                                    vb = (n * 16 + ik) * 130 + h * 65
                                    nc.tensor.matmul(
                                        po[0:64, :], V_sb[:, vb:vb + 64],
                                        pt[:, m * 512:(m + 1) * 512],
                                        start=(ik == 0), stop=(ik == 15))
                                    nc.tensor.matmul(
                                        pd[0:64, :], ones_sb[:, 0:64],
                                        pt[:, m * 512:(m + 1) * 512],
                                        start=(ik == 0), stop=(ik == 15))
                            rb = rpool.tile([64, 512], FP, tag="rb")
                            nc.vector.reciprocal(rb[:], pd[0:64, :])
                            attT_h = attT0 if h == 0 else attT1
                            nc.vector.tensor_mul(
                                attT_h[:, q0:q0 + 512], po[0:64, :], rb[:])

            # ================= AllToAll redistribution =================
            bounce_in = dram.tile([N_CORES, 128, 512], FP)
            bounce_out = dram.tile([N_CORES, 128, 512], FP)
            for j in range(NJ):
                t0 = j * 512
                nc.sync.dma_start(bounce_in[j, 0:64, :], attT0[:, t0:t0 + 512])
                nc.sync.dma_start(bounce_in[j, 64:128, :], attT1[:, t0:t0 + 512])
            if local_only:
                nc.sync.dma_start(bounce_out[:], bounce_in[:])
            else:
                nc.gpsimd.collective_compute(
                    "AllToAll", mybir.AluOpType.bypass,
                    ins=[bounce_in.opt()], outs=[bounce_out.opt()],
                    replica_groups=[list(range(N_CORES))],
                )

            # ================= Phase D: output projection =================
            with tc.tile_pool(name="psD", bufs=2, space="PSUM") as psD, \
                 tc.tile_pool(name="dsb", bufs=1) as dpool, \
                 tc.tile_pool(name="ybuf", bufs=2) as ypool:
                att_sb = []
                for i in range(NE):
                    a = dpool.tile([128, 512], FPR, tag=f"att{i}", name=f"att{i}")
                    nc.sync.dma_start(a[:], bounce_out[i].bitcast(FPR))
                    att_sb.append(a)
                wo_sb = []
                for ci in range(NE):
                    w = dpool.tile([128, EMBED], FPR, tag=f"wo{ci}", name=f"wo{ci}")
                    nc.sync.dma_start(
                        w[:], woT[ci * 128:(ci + 1) * 128, :].bitcast(FPR))
                    wo_sb.append(w)
                ao_sb = []
                for ci in range(NE):
                    a = dpool.tile([128, 32], FPR, tag=f"ao{ci}", name=f"ao{ci}")
                    nc.sync.dma_start(
                        a[:], aoT[ci * 128:(ci + 1) * 128, :].bitcast(FPR))
                    ao_sb.append(a)
                bo2_sb = dpool.tile([RANK, EMBED], FPR, tag="bo2")
                nc.sync.dma_start(bo2_sb[:], boT2.bitcast(FPR))
                bom_sb = dpool.tile([128, 8], FP, tag="bom")
                nc.sync.dma_start(bom_sb[:], bom)

                pz2 = psD.tile([32, 512], FP, tag="z2")
                for ci in range(NE):
                    nc.tensor.matmul(pz2[:], ao_sb[ci][:], att_sb[ci][:],
                                     start=(ci == 0), stop=(ci == NE - 1))
                zo_sb = dpool.tile([RANK, 512], FPR, tag="zo")
                nc.vector.tensor_copy(zo_sb[:], pz2[0:RANK, :])
                for co in range(NE):
                    py = psD.tile([128, 512], FP, tag="y")
                    for ci in range(NE):
                        nc.tensor.matmul(
                            py[:], wo_sb[ci][:, co * 128:(co + 1) * 128],
                            att_sb[ci][:], start=(ci == 0), stop=False)
                    nc.tensor.matmul(py[:], bo2_sb[:, co * 128:(co + 1) * 128],
                                     zo_sb[:], start=False, stop=True)
                    y_sb = ypool.tile([128, 512], FP, tag="y_sb")
                    nc.scalar.activation(y_sb[:], py[:], AF.Identity,
                                         bias=bom_sb[:, co:co + 1])
                    nc.sync.dma_start(Y[co * 128:(co + 1) * 128, :], y_sb[:])
    nc.compile()
    return nc


def _prep_inputs(x, Wq, bq, Aq, Bq, Wk, bk, Ak, Bk, Wv, bv, Av, Bv, Wo, bo, Ao, Bo):
    f32 = np.float32
    xTm = np.ascontiguousarray(x.reshape(T, EMBED).T.astype(f32))
    atm = np.zeros((EMBED, 96), dtype=f32)
    atm[:, 0:8] = Aq.T; atm[:, 32:40] = Ak.T; atm[:, 64:72] = Av.T
    identm = np.eye(128, dtype=f32)
    woTm = np.ascontiguousarray(Wo.T.astype(f32))
    aoTm = np.zeros((EMBED, 32), dtype=f32)
    aoTm[:, 0:8] = Ao.T
    boT2m = np.ascontiguousarray((2.0 * Bo).T.astype(f32))
    bomm = np.ascontiguousarray(bo.reshape(8, 128).T.astype(f32))
    in_maps = []
    for c in range(N_CORES):
        sl = slice(c * CH, (c + 1) * CH)
        Bm = np.zeros((128, CH), dtype=f32)
        Bm[0:8, :] = 2.0 * Bq[sl, :].T
        Bm[32:40, :] = 2.0 * Bk[sl, :].T
        Bm[64:72, :] = 2.0 * Bv[sl, :].T
        bias3m = np.stack([bq[sl], bk[sl], bv[sl]], axis=1).astype(f32)
        in_maps.append({
            "xT": xTm,
            "wqT": np.ascontiguousarray(Wq[sl, :].T.astype(f32)),
            "wkT": np.ascontiguousarray(Wk[sl, :].T.astype(f32)),
            "wvT": np.ascontiguousarray(Wv[sl, :].T.astype(f32)),
            "atQKV": atm,
            "Bqkv": np.ascontiguousarray(Bm),
            "bias3": np.ascontiguousarray(bias3m),
            "ident": identm,
            "woT": woTm,
            "aoT": aoTm,
            "boT2": boT2m,
            "bom": bomm,
            "ones64": np.ones((128, 64), dtype=f32),
        })
    return in_maps


def get_nc():
    if "nc" not in _CACHE:
        _CACHE["nc"] = _build()
    return _CACHE["nc"]


def kernel(**inputs) -> np.ndarray:
    nc = get_nc()
    in_maps = _prep_inputs(**{k: np.asarray(v) for k, v in inputs.items()})
    res = bass_utils.run_bass_kernel_spmd(
        nc, in_maps, core_ids=list(range(N_CORES)))
    yT = np.concatenate([res.results[c]["Y"] for c in range(N_CORES)], axis=1)
    return np.ascontiguousarray(yT.T).reshape(NB, S, EMBED)


if __name__ == "__main__":
    nc = get_nc()
    print("build+compile OK")



# revision 14
# speedup vs baseline: 15.0073x; 15.0073x over previous
"""LoRA self-attention Trainium2 kernel, 8-way head/tensor parallel. v2.

Sharding: core c owns heads 2c, 2c+1 (= channels 128c..128c+128) for the
QKV projections and attention; the output projection is token-sharded
(core c computes all 1024 output channels for tokens 512c..512c+512)
after an AllToAll exchange of the attention output.

Key design points (vs v1):
- LoRA folded into the weights on the host: W_eff = W + 2*B@A for q,k,v,o.
- attnV matmul fused with the softmax-denominator reduction: stationary
  operand is [ones | V_h0] / [V_h1 | ones'] so one N=512 stream yields both
  attn@V (64 rows) and the denominator replicated across 64 partitions.
  The resulting per-core channel order (h1-dims then h0-dims) is absorbed
  into a host-side permutation of Wo's contraction dim.
- P (=exp(energy)) and V in bf16; QT/KT fp32r (full-rate fp32 at N>=256).
- exp batched as [128,1024] PSUM->SBUF ACT ops; plain PSUM->SBUF copies on
  DVE (ACT stays pure-exp; ACT is the attention bottleneck).
- V^T -> V strips via ONE whole-tensor DMA-xbar transpose (bf16).
- DMA count minimized (merged 3D-AP loads) - the HWDGE trigger path
  serializes at ~625ns per dma_start.
- reps>1 repeats the body for slope-based HW timing.
"""
import sys

for p in ("/opt/trn_rl_repo",):
    if p not in sys.path:
        sys.path.append(p)

import numpy as np

import concourse.bass as bass  # noqa: F401
import concourse.tile as tile
from concourse import bacc, mybir
from concourse import bass_utils

N_CORES = 8
EMBED = 1024
HEADS = 16
HD = 64            # head dim
NB = 2             # batch
S = 2048           # seq len
T = NB * S         # 4096 tokens
CH = EMBED // N_CORES  # 128 channels (2 heads) per core
NE = EMBED // 128  # 8 contraction tiles
NJ = T // 512      # 8 token tiles of 512
NS = T // 128      # 32 token strips of 128
FP = mybir.dt.float32
FPR = mybir.dt.float32r
BF = mybir.dt.bfloat16
AF = mybir.ActivationFunctionType

_CACHE: dict = {}


def _build(local_only=False, reps=1, has_bias=False, inject_on=True, pipeline=True, epool_bufs=2):
    nc = bacc.Bacc("TRN2", target_bir_lowering=False, debug=False,
                   enable_asserts=False, num_devices=N_CORES)
    # ---- DRAM I/O (per-core) ----
    xT = nc.dram_tensor("xT", [EMBED, T], BF, kind="ExternalInput").ap()
    # packed QKV weights: row r = embed dim, cols [q(128) | k(128) | v(128)]
    wqkvT = nc.dram_tensor("wqkvT", [EMBED, 3 * CH], BF, kind="ExternalInput").ap()
    woT = nc.dram_tensor("woT", [EMBED, EMBED], BF, kind="ExternalInput").ap()
    if has_bias:
        bias3 = nc.dram_tensor("bias3", [CH, 3], FP, kind="ExternalInput").ap()
        bom = nc.dram_tensor("bom", [128, NE], FP, kind="ExternalInput").ap()
    Y = nc.dram_tensor("Y", [EMBED, 512], FP, kind="ExternalOutput").ap()

    # V_sb strip layout, bf16: per key strip s (128 tokens), base 192*s:
    #   [ones(64) | V_h0(64) | V_h1(64)]  + one trailing ones block.
    # h0 stationary = cols [192s      : 192s+128) = [O  | V0] -> [den0; att0]
    # h1 stationary = cols [192s+128  : 192s+256) = [V1 | O'] -> [att1; den1]
    VCOLS = NS * 192 + 64

    with tile.TileContext(nc) as tc, \
         nc.allow_low_precision(reason="fp32r/bf16 rounding is intentional"):
        with tc.tile_pool(name="const", bufs=1) as cpool, \
             tc.tile_pool(name="big", bufs=1) as bigpool, \
             tc.tile_pool(name="dram", bufs=1, space="DRAM") as dram:

            # packed weights: block e at cols e*384 -> [q|k|v] each [128,128]
            wqkv_sb = cpool.tile([128, NE * 384], BF, tag="wqkv")
            # wo: block ci at cols ci*1024 (all 1024 out-channels)
            wo_sb = cpool.tile([128, NE * EMBED], BF, tag="wo")
            if has_bias:
                bias_sb = cpool.tile([CH, 3], FP, tag="bias3")
                bom_sb = cpool.tile([128, NE], FP, tag="bom")

            QT_sb = bigpool.tile([CH, T], BF, tag="QT")
            KT_sb = bigpool.tile([CH, T], BF, tag="KT")
            VTb = bigpool.tile([CH, T], BF, tag="VTb")
            V_sb = bigpool.tile([128, VCOLS], BF, tag="Vstrips")

            # ones blocks (constant across reps)
            nc.vector.memset(
                V_sb[:, 0:NS * 192].rearrange("p (s c) -> p s c", c=192)[:, :, 0:64],
                1.0)
            nc.vector.memset(V_sb[:, NS * 192:VCOLS], 1.0)

            bounce_in = dram.tile([NJ, 128, 512], BF)
            bounce_out = dram.tile([NE, 128, 512], BF)

            for rep in range(reps):
                # weight (re)loads: merged single DMAs
                for h8 in range(2):
                    e0 = h8 * (NE // 2)
                    nc.sync.dma_start(
                        wqkv_sb[:, e0 * 384:(e0 + NE // 2) * 384]
                            .rearrange("p (e c) -> p e c", e=NE // 2),
                        wqkvT[e0 * 128:(e0 + NE // 2) * 128, :]
                            .rearrange("(e p) c -> p e c", p=128))
                if has_bias:
                    nc.sync.dma_start(bias_sb[:], bias3)
                    nc.sync.dma_start(bom_sb[:], bom)

                with tc.tile_pool(name="psA", bufs=2, space="PSUM") as psA, \
                     tc.tile_pool(name="xt", bufs=4) as xpool, \
                     tc.tile_pool(name="psE", bufs=epool_bufs, space="PSUM") as psE, \
                     tc.tile_pool(name="psO", bufs=1, space="PSUM") as psO, \
                     tc.tile_pool(name="pt", bufs=4) as ptpool, \
                     tc.tile_pool(name="rs", bufs=2) as rpool, \
                     tc.tile_pool(name="att", bufs=2) as apool:

                    xt_tiles = {}

                    def load_xt(j):
                        t0 = j * 512
                        xt = xpool.tile([128, NE * 512], BF, tag="xt")
                        nc.sync.dma_start(
                            xt[:].rearrange("p (e t) -> p e t", e=NE),
                            xT[:, t0:t0 + 512]
                              .rearrange("(e p) t -> p e t", p=128))
                        xt_tiles[j] = xt

                    def proj_chain_mm(j, wi, e, pp):
                        """One matmul of the (j, q/k/v) projection chain."""
                        nc.tensor.matmul(
                            pp[:],
                            wqkv_sb[:, e * 384 + wi * 128:
                                    e * 384 + wi * 128 + 128],
                            xt_tiles[j][:, e * 512:(e + 1) * 512],
                            start=(e == 0), stop=(e == NE - 1))

                    def proj_chain_out(j, wi, pp):
                        t0 = j * 512
                        dst = (QT_sb, KT_sb, VTb)[wi][:, t0:t0 + 512]
                        if has_bias:
                            nc.scalar.activation(dst, pp[:], AF.Identity,
                                                 bias=bias_sb[:, wi:wi + 1])
                        else:
                            nc.vector.tensor_copy(dst, pp[:])

                    def transpose_v(nbatch):
                        # V^T -> V strips (one DMA xbar transpose per batch):
                        # out[tok, ch] per 128-token strip at cols 192s+64.
                        s0 = nbatch * 16
                        nc.sync.dma_start_transpose(
                            V_sb[:, 192 * s0:192 * (s0 + 16)]
                                .rearrange("p (s c) -> p s c", c=192)
                                [:, :, 64:192],
                            VTb[:, 2048 * nbatch:2048 * (nbatch + 1)])

                    carry = [None, None]  # [pending_attnv, finalize]

                    def drain_carry(upto):
                        # upto=1: run prev tile's last attnv; upto=2: + finalize
                        if upto >= 1 and carry[0] is not None:
                            carry[0]()
                            carry[0] = None
                        if upto >= 2 and carry[1] is not None:
                            carry[1]()
                            carry[1] = None

                    def attn_tile(n, jq, inject, popool):
                        """Attention for query tile (n, jq); inject = list of
                        thunks interleaved into the PE stream. The last attnv
                        pair and the po normalization are deferred into the
                        next tile (cross-tile pipelining via `carry`)."""
                        j = n * 4 + jq
                        q0 = j * 512
                        if popool is psO:
                            po0 = popool.tile([128, 512], FP, tag="po0")
                            po1 = popool.tile([128, 512], FP, tag="po1")
                        else:
                            po0 = popool.tile([128, 512], FP, tag="qkv")
                            po1 = popool.tile([128, 512], FP, tag="qkv")
                        ninj = len(inject)
                        idone = 0
                        pending = None

                        def attnv(kt0, pt0, pt1):
                            first = (kt0 == n * 16)
                            last = (kt0 == n * 16 + 14)
                            for m in range(2):
                                sbase = 192 * (kt0 + m)
                                nc.tensor.matmul(
                                    po0[:], V_sb[:, sbase:sbase + 128],
                                    pt0[:, m * 512:(m + 1) * 512],
                                    start=(first and m == 0),
                                    stop=(last and m == 1))
                                nc.tensor.matmul(
                                    po1[:], V_sb[:, sbase + 128:sbase + 256],
                                    pt1[:, m * 512:(m + 1) * 512],
                                    start=(first and m == 0),
                                    stop=(last and m == 1))

                        def finalize():
                            # po0 = [den0*64 ; att0], po1 = [att1 ; den1*64]
                            rr = rpool.tile([128, 512], FP, tag="rr")
                            nc.vector.reciprocal(rr[0:HD, :], po0[0:HD, :])
                            nc.vector.reciprocal(rr[HD:128, :], po1[HD:128, :])
                            rs = rpool.tile([128, 512], FP, tag="rs")
                            # partition shift via SBUF->SBUF DMA
                            nc.sync.dma_start(rs[HD:128, :], rr[0:HD, :])
                            nc.sync.dma_start(rs[0:HD, :], rr[HD:128, :])
                            # att rows: [att1(h1 dims) ; att0(h0 dims)]
                            # - Wo is permuted on the host to match.
                            att = apool.tile([128, 512], BF, tag="att")
                            nc.vector.tensor_mul(att[0:HD, :], po1[0:HD, :],
                                                 rs[0:HD, :])
                            nc.vector.tensor_mul(att[HD:128, :], po0[HD:128, :],
                                                 rs[HD:128, :])
                            nc.sync.dma_start(bounce_in[j], att[:])

                        for g in range(8):
                            kt0 = n * 16 + 2 * g
                            E0 = psE.tile([128, 1024], FP, tag="E")
                            E1 = psE.tile([128, 1024], FP, tag="E")
                            for m in range(2):
                                k0 = (kt0 + m) * 128
                                # h0/h1 adjacent: disjoint PE row groups
                                nc.tensor.matmul(E0[:, m * 512:(m + 1) * 512],
                                                 KT_sb[0:HD, k0:k0 + 128],
                                                 QT_sb[0:HD, q0:q0 + 512],
                                                 start=True, stop=True)
                                nc.tensor.matmul(E1[:, m * 512:(m + 1) * 512],
                                                 KT_sb[HD:128, k0:k0 + 128],
                                                 QT_sb[HD:128, q0:q0 + 512],
                                                 start=True, stop=True)
                            pt0 = ptpool.tile([128, 1024], BF, tag="pt")
                            pt1 = ptpool.tile([128, 1024], BF, tag="pt")
                            nc.scalar.activation(pt0[:], E0[:], AF.Exp,
                                                 scale=0.125)
                            nc.scalar.activation(pt1[:], E1[:], AF.Exp,
                                                 scale=0.125)
                            if g == 0:
                                drain_carry(1)
                            elif g == 1:
                                drain_carry(2)
                            if pipeline:
                                if pending is not None:
                                    attnv(*pending)
                                pending = (kt0, pt0, pt1)
                            else:
                                attnv(kt0, pt0, pt1)
                            # drain injected work evenly across the 8 steps
                            want = ninj * (g + 1) // 8
                            while idone < want:
                                inject[idone]()
                                idone += 1
                        if pending is not None:
                            carry[0] = (lambda p=pending: attnv(*p))
                            carry[1] = finalize
                        else:
                            finalize()

                    # ---- Phase A for batch 0 (or all, if not injecting) ----
                    for j in range(4 if inject_on else 8):
                        load_xt(j)
                        for wi in range(3):
                            pp = psA.tile([CH, 512], FP, tag="qkv")
                            for e in range(NE):
                                proj_chain_mm(j, wi, e, pp)
                            proj_chain_out(j, wi, pp)
                    transpose_v(0)
                    if inject_on:
                        # prefetch first two b1 x-tiles so injected chains
                        # never stall the PE FIFO
                        load_xt(4)
                        load_xt(5)
                    else:
                        transpose_v(1)
                    # wo prefetch: needed only in Phase C
                    nc.sync.dma_start(
                        wo_sb[:].rearrange("p (e c) -> p e c", e=NE),
                        woT.rearrange("(e p) c -> p e c", p=128))

                    # ---- attention(batch 0) with Phase A(batch 1) injected ----
                    for jq in range(4):
                        inject = []
                        if inject_on:
                            j2 = 4 + jq
                            if j2 + 2 <= 7:
                                load_xt(j2 + 2)
                            # k first (attention(1,*) needs all of K)
                            for wi in (1, 0, 2):
                                pp = psA.tile([CH, 512], FP, tag="qkv")
                                for e in range(NE):
                                    inject.append(
                                        lambda j2=j2, wi=wi, e=e, pp=pp:
                                        proj_chain_mm(j2, wi, e, pp))
                                inject.append(
                                    lambda j2=j2, wi=wi, pp=pp:
                                    proj_chain_out(j2, wi, pp))
                        attn_tile(0, jq, inject, psO)
                    if inject_on:
                        transpose_v(1)

                    # ---- attention(batch 1) ----
                    # psA's 2 banks are idle here (no injection): alternate po
                    # accumulators between psO and psA so consecutive tiles
                    # double-buffer.
                    for jq in range(4):
                        attn_tile(1, jq, [], psA if jq % 2 else psO)
                    drain_carry(2)

                # ================= AllToAll redistribution =================
                if local_only:
                    nc.sync.dma_start(bounce_out[:], bounce_in[:])
                else:
                    nc.gpsimd.collective_compute(
                        "AllToAll", mybir.AluOpType.bypass,
                        ins=[bounce_in.opt()], outs=[bounce_out.opt()],
                        replica_groups=[list(range(N_CORES))],
                    )

                # ================= Phase C: output projection ================
                with tc.tile_pool(name="psD", bufs=2, space="PSUM") as psD, \
                     tc.tile_pool(name="dsb", bufs=1) as dpool, \
                     tc.tile_pool(name="ybuf", bufs=2) as ypool:
                    att_all = dpool.tile([128, NE * 512], BF, tag="att_all")
                    for h8 in range(2):
                        i0 = h8 * (NE // 2)
                        nc.sync.dma_start(
                            att_all[:, i0 * 512:(i0 + NE // 2) * 512]
                                .rearrange("p (i t) -> p i t", i=NE // 2),
                            bounce_out[i0:i0 + NE // 2]
                                .rearrange("i p t -> p i t"))
                    for co in range(NE):
                        py = psD.tile([128, 512], FP, tag="y")
                        for ci in range(NE):
                            nc.tensor.matmul(
                                py[:],
                                wo_sb[:, ci * EMBED + co * 128:
                                      ci * EMBED + co * 128 + 128],
                                att_all[:, ci * 512:(ci + 1) * 512],
                                start=(ci == 0), stop=(ci == NE - 1))
                        y_sb = ypool.tile([128, 512], FP, tag="y_sb")
                        if has_bias:
                            nc.scalar.activation(y_sb[:], py[:], AF.Identity,
                                                 bias=bom_sb[:, co:co + 1])
                        else:
                            nc.vector.tensor_copy(y_sb[:], py[:])
                        nc.sync.dma_start(Y[co * 128:(co + 1) * 128, :], y_sb[:])
    nc.compile()
    return nc


def _perm():
    """Per-core channel permutation produced by the attention phase:
    att chunk rows = [head(2c+1) dims, head(2c) dims]."""
    p = np.arange(EMBED).reshape(N_CORES, 2, HD)[:, ::-1, :].reshape(-1)
    return p


def _prep_inputs(x, Wq, bq, Aq, Bq, Wk, bk, Ak, Bk, Wv, bv, Av, Bv, Wo, bo, Ao, Bo):
    f32 = np.float32
    has_bias = any(np.any(np.asarray(b)) for b in (bq, bk, bv, bo))
    from ml_dtypes import bfloat16
    xTm = np.ascontiguousarray(x.reshape(T, EMBED).T.astype(bfloat16))
    # fold LoRA: W_eff = W + 2 * B @ A
    Wqe = (Wq + 2.0 * (Bq @ Aq)).astype(f32)
    Wke = (Wk + 2.0 * (Bk @ Ak)).astype(f32)
    Wve = (Wv + 2.0 * (Bv @ Av)).astype(f32)
    Woe = (Wo + 2.0 * (Bo @ Ao)).astype(f32)
    # absorb the attention-phase channel permutation into Wo's contraction dim
    woTm = np.ascontiguousarray(Woe[:, _perm()].T.astype(bfloat16))
    in_maps = []
    for c in range(N_CORES):
        sl = slice(c * CH, (c + 1) * CH)
        wqkv = np.concatenate(
            [Wqe[sl, :].T, Wke[sl, :].T, Wve[sl, :].T], axis=1).astype(bfloat16)
        m = {
            "xT": xTm,
            "wqkvT": np.ascontiguousarray(wqkv),
            "woT": woTm,
        }
        if has_bias:
            m["bias3"] = np.ascontiguousarray(
                np.stack([bq[sl], bk[sl], bv[sl]], axis=1).astype(f32))
            m["bom"] = np.ascontiguousarray(bo.reshape(NE, 128).T.astype(f32))
        in_maps.append(m)
    return has_bias, in_maps


def get_nc(local_only=False, reps=1, has_bias=False, **kw):
    key = (local_only, reps, has_bias, tuple(sorted(kw.items())))
    if key not in _CACHE:
        _CACHE[key] = _build(local_only=local_only, reps=reps,
                             has_bias=has_bias, **kw)
    return _CACHE[key]


def kernel(**inputs) -> np.ndarray:
    has_bias, in_maps = _prep_inputs(
        **{k: np.asarray(v) for k, v in inputs.items()})
    nc = get_nc(has_bias=has_bias)
    res = bass_utils.run_bass_kernel_spmd(
        nc, in_maps, core_ids=list(range(N_CORES)))
    yT = np.concatenate([res.results[c]["Y"] for c in range(N_CORES)], axis=1)
    return np.ascontiguousarray(yT.T).reshape(NB, S, EMBED)


if __name__ == "__main__":
    nc = get_nc()
    print("build+compile OK")


# revision 23
# speedup vs baseline: 47.7710x; 3.1832x over previous
"""LoRA self-attention Trainium2 kernel, 8-way head/tensor parallel. v2.

Sharding: core c owns heads 2c, 2c+1 (= channels 128c..128c+128) for the
QKV projections and attention; the output projection is token-sharded
(core c computes all 1024 output channels for tokens 512c..512c+512)
after an AllToAll exchange of the attention output.

Key design points (vs v1):
- LoRA folded into the weights on the host: W_eff = W + 2*B@A for q,k,v,o.
- attnV matmul fused with the softmax-denominator reduction: stationary
  operand is [ones | V_h0] / [V_h1 | ones'] so one N=512 stream yields both
  attn@V (64 rows) and the denominator replicated across 64 partitions.
  The resulting per-core channel order (h1-dims then h0-dims) is absorbed
  into a host-side permutation of Wo's contraction dim.
- P (=exp(energy)) and V in bf16; QT/KT fp32r (full-rate fp32 at N>=256).
- exp batched as [128,1024] PSUM->SBUF ACT ops; plain PSUM->SBUF copies on
  DVE (ACT stays pure-exp; ACT is the attention bottleneck).
- V^T -> V strips via ONE whole-tensor DMA-xbar transpose (bf16).
- DMA count minimized (merged 3D-AP loads) - the HWDGE trigger path
  serializes at ~625ns per dma_start.
- reps>1 repeats the body for slope-based HW timing.
"""
import sys

for p in ("/opt/trn_rl_repo",):
    if p not in sys.path:
        sys.path.append(p)

import numpy as np

import concourse.bass as bass  # noqa: F401
import concourse.tile as tile
from concourse import bacc, mybir
from concourse import bass_utils

N_CORES = 8
EMBED = 1024
HEADS = 16
HD = 64            # head dim
NB = 2             # batch
S = 2048           # seq len
T = NB * S         # 4096 tokens
CH = EMBED // N_CORES  # 128 channels (2 heads) per core
NE = EMBED // 128  # 8 contraction tiles
NJ = T // 512      # 8 token tiles of 512
NS = T // 128      # 32 token strips of 128
FP = mybir.dt.float32
FPR = mybir.dt.float32r
BF = mybir.dt.bfloat16
F8 = mybir.dt.float8e4
AF = mybir.ActivationFunctionType

_CACHE: dict = {}


def _build(local_only=False, reps=1, has_bias=False, inject_on=True, pipeline=True, epool_bufs=2, fp8_attnv=True):
    nc = bacc.Bacc("TRN2", target_bir_lowering=False, debug=False,
                   enable_asserts=False, num_devices=N_CORES)
    # ---- DRAM I/O (per-core) ----
    xT = nc.dram_tensor("xT", [EMBED, T], BF, kind="ExternalInput").ap()
    # packed QKV weights: row r = embed dim, cols [q(128) | k(128) | v(128)]
    wqkvT = nc.dram_tensor("wqkvT", [EMBED, 3 * CH], BF, kind="ExternalInput").ap()
    woT = nc.dram_tensor("woT", [EMBED, EMBED], BF, kind="ExternalInput").ap()
    if has_bias:
        bias3 = nc.dram_tensor("bias3", [CH, 3], FP, kind="ExternalInput").ap()
        bom = nc.dram_tensor("bom", [128, NE], FP, kind="ExternalInput").ap()
    Y = nc.dram_tensor("Y", [EMBED, 512], FP, kind="ExternalOutput").ap()

    # V_sb strip layout, bf16: per key strip s (128 tokens), base 192*s:
    #   [ones(64) | V_h0(64) | V_h1(64)]  + one trailing ones block.
    # h0 stationary = cols [192s      : 192s+128) = [O  | V0] -> [den0; att0]
    # h1 stationary = cols [192s+128  : 192s+256) = [V1 | O'] -> [att1; den1]
    VCOLS = NS * 192 + 128

    with tile.TileContext(nc) as tc, \
         nc.allow_low_precision(reason="fp32r/bf16 rounding is intentional"):
        with tc.tile_pool(name="const", bufs=1) as cpool, \
             tc.tile_pool(name="big", bufs=1) as bigpool, \
             tc.tile_pool(name="dram", bufs=1, space="DRAM") as dram:

            # packed weights: block e at cols e*384 -> [q|k|v] each [128,128]
            wqkv_sb = cpool.tile([128, NE * 384], BF, tag="wqkv")
            # wo: block ci at cols ci*1024 (all 1024 out-channels)
            wo_sb = cpool.tile([128, NE * EMBED], BF, tag="wo")
            if has_bias:
                bias_sb = cpool.tile([CH, 3], FP, tag="bias3")
                bom_sb = cpool.tile([128, NE], FP, tag="bom")

            QT_sb = bigpool.tile([CH, T], BF, tag="QT")
            KT_sb = bigpool.tile([CH, T], BF, tag="KT")
            VTb = bigpool.tile([CH, T], BF, tag="VTb")
            VDT = F8 if fp8_attnv else BF
            V_sb = bigpool.tile([128, VCOLS], VDT, tag="Vstrips")
            if fp8_attnv:
                # bf16 staging for the xbar transpose (xbar needs 2-byte)
                V_bf = bigpool.tile([128, VCOLS], BF, tag="Vbf")

            # ones blocks (constant across reps)
            nc.vector.memset(
                V_sb[:, 0:NS * 192].rearrange("p (s c) -> p s c", c=192)[:, :, 0:64],
                1.0)
            nc.vector.memset(V_sb[:, NS * 192:VCOLS], 1.0)

            bounce_in = dram.tile([NJ, 128, 512], BF)
            bounce_out = dram.tile([NE, 128, 512], BF)

            for rep in range(reps):
                # weight (re)loads: merged single DMAs
                for h8 in range(2):
                    e0 = h8 * (NE // 2)
                    nc.sync.dma_start(
                        wqkv_sb[:, e0 * 384:(e0 + NE // 2) * 384]
                            .rearrange("p (e c) -> p e c", e=NE // 2),
                        wqkvT[e0 * 128:(e0 + NE // 2) * 128, :]
                            .rearrange("(e p) c -> p e c", p=128))
                if has_bias:
                    nc.sync.dma_start(bias_sb[:], bias3)
                    nc.sync.dma_start(bom_sb[:], bom)

                with tc.tile_pool(name="psA", bufs=2, space="PSUM") as psA, \
                     tc.tile_pool(name="xt", bufs=6) as xpool, \
                     tc.tile_pool(name="psE", bufs=epool_bufs, space="PSUM") as psE, \
                     tc.tile_pool(name="psO", bufs=1, space="PSUM") as psO, \
                     tc.tile_pool(name="pt", bufs=4) as ptpool, \
                     tc.tile_pool(name="rs", bufs=2) as rpool, \
                     tc.tile_pool(name="att", bufs=2) as apool:

                    xt_tiles = {}

                    def load_xt(j):
                        t0 = j * 512
                        xt = xpool.tile([128, NE * 512], BF, tag="xt")
                        nc.sync.dma_start(
                            xt[:].rearrange("p (e t) -> p e t", e=NE),
                            xT[:, t0:t0 + 512]
                              .rearrange("(e p) t -> p e t", p=128))
                        xt_tiles[j] = xt

                    def proj_chain_mm(j, wi, e, pp):
                        """One matmul of the (j, q/k/v) projection chain."""
                        nc.tensor.matmul(
                            pp[:],
                            wqkv_sb[:, e * 384 + wi * 128:
                                    e * 384 + wi * 128 + 128],
                            xt_tiles[j][:, e * 512:(e + 1) * 512],
                            start=(e == 0), stop=(e == NE - 1))

                    def proj_chain_out(j, wi, pp):
                        t0 = j * 512
                        dst = (QT_sb, KT_sb, VTb)[wi][:, t0:t0 + 512]
                        if has_bias:
                            nc.scalar.activation(dst, pp[:], AF.Identity,
                                                 bias=bias_sb[:, wi:wi + 1])
                        else:
                            nc.vector.tensor_copy(dst, pp[:])

                    def transpose_v(nbatch):
                        # V^T -> V strips (one DMA xbar transpose per batch):
                        # out[tok, ch] per 128-token strip at cols 192s+64.
                        s0 = nbatch * 16
                        vdst = V_bf if fp8_attnv else V_sb
                        nc.sync.dma_start_transpose(
                            vdst[:, 192 * s0:192 * (s0 + 16)]
                                .rearrange("p (s c) -> p s c", c=192)
                                [:, :, 64:192],
                            VTb[:, 2048 * nbatch:2048 * (nbatch + 1)])
                        if fp8_attnv:
                            # DVE cast of the V regions into the fp8 strips
                            nc.vector.tensor_copy(
                                V_sb[:, 192 * s0:192 * (s0 + 16)]
                                    .rearrange("p (s c) -> p s c", c=192)
                                    [:, :, 64:192],
                                V_bf[:, 192 * s0:192 * (s0 + 16)]
                                    .rearrange("p (s c) -> p s c", c=192)
                                    [:, :, 64:192])

                    carry = [None, None]  # [pending_attnv, finalize]

                    def drain_carry(upto):
                        # upto=1: run prev tile's last attnv; upto=2: + finalize
                        if upto >= 1 and carry[0] is not None:
                            carry[0]()
                            carry[0] = None
                        if upto >= 2 and carry[1] is not None:
                            carry[1]()
                            carry[1] = None

                    def attn_tile(n, jq, inject, popool):
                        """Attention for query tile (n, jq); inject = list of
                        thunks interleaved into the PE stream. The last attnv
                        pair and the po normalization are deferred into the
                        next tile (cross-tile pipelining via `carry`)."""
                        j = n * 4 + jq
                        q0 = j * 512
                        if popool is psO:
                            po0 = popool.tile([128, 512], FP, tag="po0")
                            po1 = popool.tile([128, 512], FP, tag="po1")
                        else:
                            po0 = popool.tile([128, 512], FP, tag="qkv")
                            po1 = popool.tile([128, 512], FP, tag="qkv")
                        ninj = len(inject)
                        idone = 0
                        pending = None

                        def attnv(kt0, pt0, pt1):
                            first = (kt0 == n * 16)
                            last = (kt0 == n * 16 + 14)
                            sbase = 192 * kt0
                            if fp8_attnv:
                                # DoubleRow: one MM per head covers both key
                                # strips (ko dim pairs strip kt0 / kt0+1)
                                for po, coff, pt in ((po0, 0, pt0),
                                                    (po1, 128, pt1)):
                                    nc.tensor.matmul(
                                        po[:],
                                        V_sb[:, sbase + coff:sbase + coff + 384]
                                            .rearrange("p (ko m) -> p ko m",
                                                       ko=2)[:, :, 0:128],
                                        pt[:].rearrange("p (ko t) -> p ko t",
                                                        ko=2),
                                        start=first, stop=last,
                                        perf_mode=mybir.MatmulPerfMode.DoubleRow)
                            else:
                                for m in range(2):
                                    sb = sbase + 192 * m
                                    nc.tensor.matmul(
                                        po0[:], V_sb[:, sb:sb + 128],
                                        pt0[:, m * 512:(m + 1) * 512],
                                        start=(first and m == 0),
                                        stop=(last and m == 1))
                                    nc.tensor.matmul(
                                        po1[:], V_sb[:, sb + 128:sb + 256],
                                        pt1[:, m * 512:(m + 1) * 512],
                                        start=(first and m == 0),
                                        stop=(last and m == 1))

                        def finalize():
                            # po0 = [den0*64 ; att0], po1 = [att1 ; den1*64]
                            rr = rpool.tile([128, 512], FP, tag="rr")
                            nc.vector.reciprocal(rr[0:HD, :], po0[0:HD, :])
                            nc.vector.reciprocal(rr[HD:128, :], po1[HD:128, :])
                            rs = rpool.tile([128, 512], FP, tag="rs")
                            # partition shift via SBUF->SBUF DMA
                            nc.sync.dma_start(rs[HD:128, :], rr[0:HD, :])
                            nc.sync.dma_start(rs[0:HD, :], rr[HD:128, :])
                            # att rows: [att1(h1 dims) ; att0(h0 dims)]
                            # - Wo is permuted on the host to match.
                            att = apool.tile([128, 512], BF, tag="att")
                            nc.vector.tensor_mul(att[0:HD, :], po1[0:HD, :],
                                                 rs[0:HD, :])
                            nc.vector.tensor_mul(att[HD:128, :], po0[HD:128, :],
                                                 rs[HD:128, :])
                            nc.sync.dma_start(bounce_in[j], att[:])

                        for g in range(8):
                            kt0 = n * 16 + 2 * g
                            E0 = psE.tile([128, 1024], FP, tag="E")
                            E1 = psE.tile([128, 1024], FP, tag="E")
                            for m in range(2):
                                k0 = (kt0 + m) * 128
                                # h0/h1 adjacent: disjoint PE row groups
                                nc.tensor.matmul(E0[:, m * 512:(m + 1) * 512],
                                                 KT_sb[0:HD, k0:k0 + 128],
                                                 QT_sb[0:HD, q0:q0 + 512],
                                                 start=True, stop=True)
                                nc.tensor.matmul(E1[:, m * 512:(m + 1) * 512],
                                                 KT_sb[HD:128, k0:k0 + 128],
                                                 QT_sb[HD:128, q0:q0 + 512],
                                                 start=True, stop=True)
                            pt0 = ptpool.tile([128, 1024], VDT, tag="pt")
                            pt1 = ptpool.tile([128, 1024], VDT, tag="pt")
                            nc.scalar.activation(pt0[:], E0[:], AF.Exp,
                                                 scale=0.125)
                            nc.scalar.activation(pt1[:], E1[:], AF.Exp,
                                                 scale=0.125)
                            if g == 0:
                                drain_carry(1)
                            elif g == 1:
                                drain_carry(2)
                            if pipeline:
                                if pending is not None:
                                    attnv(*pending)
                                pending = (kt0, pt0, pt1)
                            else:
                                attnv(kt0, pt0, pt1)
                            # drain injected work evenly across the 8 steps
                            want = ninj * (g + 1) // 8
                            while idone < want:
                                inject[idone]()
                                idone += 1
                        if pending is not None:
                            carry[0] = (lambda p=pending: attnv(*p))
                            carry[1] = finalize
                        else:
                            finalize()

                    # ---- Phase A for batch 0 (or all, if not injecting) ----
                    for j in range(4 if inject_on else 8):
                        load_xt(j)
                        for wi in range(3):
                            pp = psA.tile([CH, 512], FP, tag="qkv")
                            for e in range(NE):
                                proj_chain_mm(j, wi, e, pp)
                            proj_chain_out(j, wi, pp)
                    transpose_v(0)
                    if inject_on:
                        # prefetch first two b1 x-tiles so injected chains
                        # never stall the PE FIFO
                        load_xt(4)
                        load_xt(5)
                    else:
                        transpose_v(1)
                    # wo prefetch: needed only in Phase C
                    nc.sync.dma_start(
                        wo_sb[:].rearrange("p (e c) -> p e c", e=NE),
                        woT.rearrange("(e p) c -> p e c", p=128))

                    # ---- attention(batch 0) with Phase A(batch 1) injected ----
                    # all 12 batch-1 projection chains packed into the
                    # first three b0 tiles (4 per tile): tile (0,3) then runs
                    # ACT-paced like batch 1, rebalancing PE work.
                    chains = []
                    if inject_on:
                        for j2 in range(4, 8):
                            # k first within each token tile
                            for wi in (1, 0, 2):
                                chains.append((j2, wi))
                    if inject_on:
                        load_xt(4)
                        load_xt(5)
                    for jq in range(4):
                        inject = []
                        if inject_on:
                            if jq + 6 <= 7:
                                load_xt(jq + 6)
                            for j2, wi in chains[jq * 3:(jq + 1) * 3]:
                                pp = psA.tile([CH, 512], FP, tag="qkv")
                                for e in range(NE):
                                    inject.append(
                                        lambda j2=j2, wi=wi, e=e, pp=pp:
                                        proj_chain_mm(j2, wi, e, pp))
                                inject.append(
                                    lambda j2=j2, wi=wi, pp=pp:
                                    proj_chain_out(j2, wi, pp))
                        attn_tile(0, jq, inject, psO)
                    if inject_on:
                        transpose_v(1)

                    # ---- attention(batch 1) ----
                    # psA's 2 banks are idle here (no injection): alternate po
                    # accumulators between psO and psA so consecutive tiles
                    # double-buffer.
                    for jq in range(4):
                        attn_tile(1, jq, [], psA if jq % 2 else psO)
                    drain_carry(2)

                # ================= AllToAll redistribution =================
                if local_only:
                    nc.sync.dma_start(bounce_out[:], bounce_in[:])
                else:
                    nc.gpsimd.collective_compute(
                        "AllToAll", mybir.AluOpType.bypass,
                        ins=[bounce_in.opt()], outs=[bounce_out.opt()],
                        replica_groups=[list(range(N_CORES))],
                    )

                # ================= Phase C: output projection ================
                with tc.tile_pool(name="psD", bufs=2, space="PSUM") as psD, \
                     tc.tile_pool(name="dsb", bufs=1) as dpool, \
                     tc.tile_pool(name="ybuf", bufs=2) as ypool:
                    att_all = dpool.tile([128, NE * 512], BF, tag="att_all")
                    for h8 in range(2):
                        i0 = h8 * (NE // 2)
                        nc.sync.dma_start(
                            att_all[:, i0 * 512:(i0 + NE // 2) * 512]
                                .rearrange("p (i t) -> p i t", i=NE // 2),
                            bounce_out[i0:i0 + NE // 2]
                                .rearrange("i p t -> p i t"))
                    for co in range(NE):
                        py = psD.tile([128, 512], FP, tag="y")
                        for ci in range(NE):
                            nc.tensor.matmul(
                                py[:],
                                wo_sb[:, ci * EMBED + co * 128:
                                      ci * EMBED + co * 128 + 128],
                                att_all[:, ci * 512:(ci + 1) * 512],
                                start=(ci == 0), stop=(ci == NE - 1))
                        y_sb = ypool.tile([128, 512], FP, tag="y_sb")
                        if has_bias:
                            nc.scalar.activation(y_sb[:], py[:], AF.Identity,
                                                 bias=bom_sb[:, co:co + 1])
                        else:
                            nc.vector.tensor_copy(y_sb[:], py[:])
                        nc.sync.dma_start(Y[co * 128:(co + 1) * 128, :], y_sb[:])
    nc.compile()
    return nc


def _perm():
    """Per-core channel permutation produced by the attention phase:
    att chunk rows = [head(2c+1) dims, head(2c) dims]."""
    p = np.arange(EMBED).reshape(N_CORES, 2, HD)[:, ::-1, :].reshape(-1)
    return p


def _prep_inputs(x, Wq, bq, Aq, Bq, Wk, bk, Ak, Bk, Wv, bv, Av, Bv, Wo, bo, Ao, Bo):
    f32 = np.float32
    has_bias = any(np.any(np.asarray(b)) for b in (bq, bk, bv, bo))
    from ml_dtypes import bfloat16
    xTm = np.ascontiguousarray(x.reshape(T, EMBED).T.astype(bfloat16))
    # fold LoRA: W_eff = W + 2 * B @ A
    Wqe = (Wq + 2.0 * (Bq @ Aq)).astype(f32)
    Wke = (Wk + 2.0 * (Bk @ Ak)).astype(f32)
    Wve = (Wv + 2.0 * (Bv @ Av)).astype(f32)
    Woe = (Wo + 2.0 * (Bo @ Ao)).astype(f32)
    # absorb the attention-phase channel permutation into Wo's contraction dim
    woTm = np.ascontiguousarray(Woe[:, _perm()].T.astype(bfloat16))
    in_maps = []
    for c in range(N_CORES):
        sl = slice(c * CH, (c + 1) * CH)
        wqkv = np.concatenate(
            [Wqe[sl, :].T, Wke[sl, :].T, Wve[sl, :].T], axis=1).astype(bfloat16)
        m = {
            "xT": xTm,
            "wqkvT": np.ascontiguousarray(wqkv),
            "woT": woTm,
        }
        if has_bias:
            m["bias3"] = np.ascontiguousarray(
                np.stack([bq[sl], bk[sl], bv[sl]], axis=1).astype(f32))
            m["bom"] = np.ascontiguousarray(bo.reshape(NE, 128).T.astype(f32))
        in_maps.append(m)
    return has_bias, in_maps


def get_nc(local_only=False, reps=1, has_bias=False, **kw):
    key = (local_only, reps, has_bias, tuple(sorted(kw.items())))
    if key not in _CACHE:
        _CACHE[key] = _build(local_only=local_only, reps=reps,
                             has_bias=has_bias, **kw)
    return _CACHE[key]


def kernel(**inputs) -> np.ndarray:
    has_bias, in_maps = _prep_inputs(
        **{k: np.asarray(v) for k, v in inputs.items()})
    nc = get_nc(has_bias=has_bias)
    res = bass_utils.run_bass_kernel_spmd(
        nc, in_maps, core_ids=list(range(N_CORES)))
    yT = np.concatenate([res.results[c]["Y"] for c in range(N_CORES)], axis=1)
    return np.ascontiguousarray(yT.T).reshape(NB, S, EMBED)


if __name__ == "__main__":
    nc = get_nc()
    print("build+compile OK")
